# revision 20
# baseline (speedup 1.0000x reference)
"""AbstractContrastiveLoss on 8 TRN2 NeuronCores.

Data-parallel over (sample, half) -> 8 shards. Gather-free formulation:
all per-pixel class-dependent terms flow through TWO bf16 matmuls against
a host-precomputed one-hot (shipped in BOTH layouts so neither matmul
needs per-chunk transposes), with e shipped PRE-TRANSPOSED (F, 16) and
pre-converted to bf16 so both matmuls contract/produce along the
pixel-major axis and the distance tail is a pure free-axis bf16 reduce:
  forward : sums  = oh_pm^T @ e_bf16       (C, 16)  segment sums
  backward: g_aug = ohc_cm^T @ [mu|w|0]    (F, 32)  per-pixel gather of
            mu[t] and 1/count[t] in ONE bf16 matmul (32-wide output
            keeps the result slices coalesced off ScalarE).
  d^2     = sum((e_bf - g_bf)^2, axis=-1)  free-axis reduce, no transpose,
            no batched per-pixel dot, no cross-partition reduction.
counts (and 1/counts) are pure functions of the int target and are
precomputed on host, so no on-device one-hot build and no int gathers.
Cross-shard reduction of sums via psum pairs; final mean on host.
"""

import numpy as np

N, E, C = 4, 16, 64
H = W = 768
P = H * W
F = P // 2
DELTA_VAR = 0.5
DELTA_DIST = 2.0
ALPHA, BETA, GAMMA = 1.0, 1.0, 0.001
EPS = 1e-12

_CACHE = {}


def _build():
    import jax
    import jax.numpy as jnp
    from jax.sharding import Mesh, PartitionSpec as PS
    from jax.experimental.shard_map import shard_map

    devs = jax.devices()[:8]
    mesh = Mesh(np.asarray(devs), ("c",))

    def shard_fn(et_bf, oh, ohc, winv):
        # et_bf: (1, F, 16) bf16  this core's half-sample, PIXEL-major
        #        (host pre-converts: same rounding as an on-device cast,
        #        but halves the upload and kills the convert pass)
        # oh:    (1, F, C)  bf16  host-built one-hot, pixel-major
        # ohc:   (1, C, F)  bf16  the same one-hot, CLASS-major
        # winv:  (1, C)     f32   1/max(counts,1) for this core's sample
        et_bf = et_bf[0]
        oh = oh[0]
        ohc = ohc[0]
        winv = winv[0]

        # forward: per-class segment sums; contraction over pixels, both
        # operands pixel-major -> clean PE lowering, f32 accumulation.
        sums = jnp.einsum(
            "fc,fj->cj",
            oh,
            et_bf,
            preferred_element_type=jnp.float32,
        )  # (C, 16)
        red = jax.lax.psum(
            sums, "c", axis_index_groups=[[0, 1], [2, 3], [4, 5], [6, 7]]
        )
        mu = red * winv[:, None]  # (C, 16)

        # backward: fused gather of [mu | w] via one bf16 matmul. Using the
        # CLASS-major one-hot here makes each (64, 128) lhsT chunk land in
        # the PE without a per-chunk transpose (the pixel-major one-hot
        # would need 2304 identity-matmul transposes).
        # 32 columns [mu | w | zeros]: bf16 output halves PSUM/transpose
        # bytes vs f32, and the 32-wide tile keeps the slice/convert of the
        # result coalesced (an 18-wide bf16 output routed ~3850 small-tile
        # converts to ScalarE; 32-wide keeps ACT at ~990 instructions).
        M = jnp.concatenate(
            [mu, winv[:, None], jnp.zeros((C, 32 - E - 1), mu.dtype)], axis=1
        )  # (C, 32)
        g_aug = jnp.einsum(
            "cf,cj->fj", ohc, M.astype(jnp.bfloat16),
            preferred_element_type=jnp.bfloat16,
        )  # (F, 32) bf16
        w_t = g_aug[:, E].astype(jnp.float32)     # 1/count[t] (F,)

        # d^2 via direct difference in bf16 (reuses et_bf from the forward
        # matmul: halves DVE mode cost + 9.4MB of f32 reads), f32 reduce.
        diff = et_bf - g_aug[:, :E]
        d2 = jnp.sum(jnp.square(diff), axis=1, dtype=jnp.float32)  # (F,)
        d = jnp.sqrt(d2 + EPS)
        h = jnp.maximum(d - DELTA_VAR, 0.0)
        var_part = jnp.sum(h * h * w_t)

        # distance + regularizer terms on (C, C): tiny, f32
        sq = jnp.sum(mu * mu, axis=1)  # (C,)
        gram = mu @ mu.T
        dmat = jnp.sqrt(
            jnp.maximum(sq[:, None] + sq[None, :] - 2 * gram, 0.0) + EPS
        )
        rep = 2.0 * DELTA_DIST * (1.0 - jnp.eye(C, dtype=jnp.float32))
        dist = jnp.sum(jnp.maximum(rep - dmat, 0.0) ** 2)
        reg = jnp.sum(jnp.sqrt(sq + EPS))
        out = jnp.stack([var_part, dist, reg])
        return out[None, :]  # (1, 3)

    fn = shard_map(
        shard_fn, mesh=mesh,
        in_specs=(PS("c"), PS("c"), PS("c"), PS("c")),
        out_specs=PS("c"),
        check_rep=False,
    )
    from jax.sharding import NamedSharding

    return jax.jit(fn), NamedSharding(mesh, PS("c"))


def _prep(input_, target):
    """Host-side shard + transpose + one-hot precompute (layout/int work)."""
    import ml_dtypes

    bf16 = np.dtype(ml_dtypes.bfloat16)
    # (8, F, 16) pixel-major shards, pre-converted to bf16 on host
    e8 = np.empty((8, F, E), dtype=bf16)
    for c in range(8):
        e8[c] = (
            input_[c // 2]
            .reshape(E, P)[:, (c % 2) * F: (c % 2 + 1) * F]
            .T.astype(bf16)
        )
    t8 = np.stack(
        [
            target[c // 2].reshape(P)[(c % 2) * F: (c % 2 + 1) * F]
            for c in range(8)
        ]
    )
    # one-hot in bf16 (exact 0/1), in both layouts, via direct scatter-fill
    # (avoids two 150MB bool-compare temporaries + device-side casts)
    fidx = np.arange(F)
    oh8 = np.zeros((8, F, C), dtype=bf16)                        # (8, F, C)
    ohc8 = np.zeros((8, C, F), dtype=bf16)                       # (8, C, F)
    one = np.ones((), dtype=bf16)
    for c in range(8):
        oh8[c, fidx, t8[c]] = one
        ohc8[c, t8[c], fidx] = one
    # per-sample counts -> per-core 1/max(counts,1)
    w8 = np.empty((8, C), dtype=np.float32)
    for n in range(N):
        cnt = np.bincount(target[n].reshape(-1), minlength=C).astype(np.float32)
        winv = 1.0 / np.maximum(cnt, 1.0)
        w8[2 * n] = winv
        w8[2 * n + 1] = winv
    return e8, oh8, ohc8, w8


def kernel(input_, target):
    import jax.numpy as jnp

    input_ = np.ascontiguousarray(np.asarray(input_, dtype=np.float32))
    target = np.ascontiguousarray(np.asarray(target, dtype=np.int32))

    if "fn" not in _CACHE:
        _CACHE["fn"], _CACHE["sh"] = _build()
    fn, sh = _CACHE["fn"], _CACHE["sh"]

    e8, oh8, ohc8, w8 = _prep(input_, target)
    # upload each core's shard directly to its device (avoids committing
    # ~680MB to device 0 and resharding through _multi_slice programs)
    import jax

    out = None
    for attempt in range(3):
        try:
            ej, ohj, ohcj, wj = (
                jax.device_put(x, sh) for x in (e8, oh8, ohc8, w8)
            )
            out = np.asarray(fn(ej, ohj, ohcj, wj))  # (8, 3)
            break
        except Exception:
            # transient device failures (worker hangup / NRT exec-unit
            # unrecoverable) recover on retry; re-upload since arrays on
            # a crashed device may be invalid
            if attempt == 2:
                raise

    loss = 0.0
    for n in range(N):
        a, b = out[2 * n], out[2 * n + 1]
        var = (float(a[0]) + float(b[0])) / C
        dist = float(a[1]) / (C * (C - 1))
        reg = float(a[2]) / C
        loss += ALPHA * var + BETA * dist + GAMMA * reg
    return np.float32(loss / N)
